# revision 1
# baseline (speedup 1.0000x reference)
"""Data-parallel DeepIce transformer kernel for 8 Trainium2 NeuronCores.

Sharding: batch B=8 is split 1 element per core (rel_pos_bias, attention
and MLP are all batch-independent). Weights are broadcast to every core.
Each core runs the full 16-layer network for its batch element; the [B, 3]
output is gathered on host.
"""

import math
from functools import partial

import jax
import jax.numpy as jnp
import numpy as np

DIM = 384
HEADS = 12
HD = 32
L_REL = 4
L_STD = 12
MLP_H = 4 * DIM
B = 8
N = 256
N_REL = 1
TIME_SCALE = 30000.0 / 500 * 0.3
SCALE = HD ** -0.5

_NCORES = 8


def _ln(x, g, b):
    m = x.mean(-1, keepdims=True)
    v = ((x - m) ** 2).mean(-1, keepdims=True)
    return (x - m) / jnp.sqrt(v + 1e-5) * g + b


def _sin_emb(x, dim):
    half = dim // 2
    freq = jnp.exp(jnp.arange(half, dtype=x.dtype) * (-math.log(10000.0) / half))
    e = x[..., None] * freq
    return jnp.concatenate([jnp.sin(e), jnp.cos(e)], -1)


def _rel_bias(x0, w1d, b1d, wp, bp):
    pos = x0[..., :3]
    t = x0[..., 3:4]
    dpos = pos[:, :, None] - pos[:, None, :]
    dt = t[:, :, None] - t[:, None, :] * TIME_SCALE
    dpos2 = dpos ** 2
    dt2 = dt ** 2
    ds2 = dpos2.sum(-1) - dt2[..., 0]
    d = jnp.sign(ds2) * jnp.sqrt(jnp.abs(ds2))
    f = jnp.concatenate([dpos, dt, dpos2, dt2], -1)
    d = d + (f @ w1d.T + b1d)[..., 0]
    emb = _sin_emb(1024.0 * jnp.clip(d, -4.0, 4.0), HD)
    return emb @ wp.T + bp


def _mask_bias(mask):
    mn = jnp.minimum(mask[:, None, :], mask[:, :, None])
    mx = jnp.maximum(mask[:, None, :], mask[:, :, None])
    return jnp.where(mx < 0, 0.0, mn)


def _attn(xq, xkv, wq, wk, wv, wo, bo, rel, mbias):
    Bq, Nq, _ = xq.shape
    q = (xq @ wq.T).reshape(Bq, Nq, HEADS, HD).transpose(0, 2, 1, 3) * SCALE
    k = (xkv @ wk.T).reshape(Bq, Nq, HEADS, HD).transpose(0, 2, 1, 3)
    v = (xkv @ wv.T).reshape(Bq, Nq, HEADS, HD).transpose(0, 2, 1, 3)
    a = jnp.einsum('bhic,bhjc->bhij', q, k)
    if rel is not None:
        a = a + jnp.einsum('bhic,bijc->bhij', q, rel)
    if mbias is not None:
        a = a + mbias[:, None]
    a = jax.nn.softmax(a, axis=-1)
    o = jnp.einsum('bhij,bhjc->bihc', a, v)
    if rel is not None:
        o = o + jnp.einsum('bhij,bijc->bihc', a, rel)
    return o.reshape(Bq, Nq, HEADS * HD) @ wo.T + bo


def _mlp(x, w1, b1, w2, b2):
    return jax.nn.gelu(x @ w1.T + b1, approximate=False) @ w2.T + b2


def _forward(x, x0, mask, rp_w1d, rp_b1d, rp_wp, rp_bp,
             sw_ln1_g, sw_ln1_b, sw_ln2_g, sw_ln2_b, sw_wq, sw_wk, sw_wv, sw_wo, sw_bo,
             sw_w1, sw_b1, sw_w2, sw_b2,
             bl_ln1_g, bl_ln1_b, bl_ln2_g, bl_ln2_b, bl_wq, bl_wk, bl_wv, bl_wo, bl_bo,
             bl_w1, bl_b1, bl_w2, bl_b2, bl_g1, bl_g2, cls_w, out_w, out_b):
    # x: [1, N, DIM] on this core (one batch element)
    rel = _rel_bias(x0, rp_w1d, rp_b1d, rp_wp, rp_bp)
    mbias = _mask_bias(mask)
    for i in range(L_REL):
        r = rel if i < N_REL else None
        xn = _ln(x, sw_ln1_g[i], sw_ln1_b[i])
        x = x + _attn(xn, xn, sw_wq[i], sw_wk[i], sw_wv[i], sw_wo[i], sw_bo[i], r, mbias)
        x = x + _mlp(_ln(x, sw_ln2_g[i], sw_ln2_b[i]), sw_w1[i], sw_b1[i], sw_w2[i], sw_b2[i])
    cls = jnp.broadcast_to(cls_w[None, None, :], (x.shape[0], 1, DIM))
    x = jnp.concatenate([cls, x], axis=1)
    mask2 = jnp.concatenate([jnp.zeros((x.shape[0], 1), mask.dtype), mask], axis=1)
    mbias2 = _mask_bias(mask2)
    for i in range(L_STD):
        xn = _ln(x, bl_ln1_g[i], bl_ln1_b[i])
        x = x + bl_g1[i] * _attn(xn, xn, bl_wq[i], bl_wk[i], bl_wv[i], bl_wo[i], bl_bo[i], None, mbias2)
        x = x + bl_g2[i] * _mlp(_ln(x, bl_ln2_g[i], bl_ln2_b[i]), bl_w1[i], bl_b1[i], bl_w2[i], bl_b2[i])
    return x[:, 0] @ out_w.T + out_b


_ARG_NAMES = [
    'x', 'x0', 'mask', 'rp_w1d', 'rp_b1d', 'rp_wp', 'rp_bp',
    'sw_ln1_g', 'sw_ln1_b', 'sw_ln2_g', 'sw_ln2_b', 'sw_wq', 'sw_wk', 'sw_wv',
    'sw_wo', 'sw_bo', 'sw_w1', 'sw_b1', 'sw_w2', 'sw_b2',
    'bl_ln1_g', 'bl_ln1_b', 'bl_ln2_g', 'bl_ln2_b', 'bl_wq', 'bl_wk', 'bl_wv',
    'bl_wo', 'bl_bo', 'bl_w1', 'bl_b1', 'bl_w2', 'bl_b2', 'bl_g1', 'bl_g2',
    'cls_w', 'out_w', 'out_b',
]

_pmapped = None


def _get_pmapped():
    global _pmapped
    if _pmapped is None:
        # x, x0, mask are sharded over the leading (batch) axis; everything
        # else is broadcast to all 8 cores.
        in_axes = (0, 0, 0) + (None,) * (len(_ARG_NAMES) - 3)
        _pmapped = jax.pmap(_forward, in_axes=in_axes, devices=jax.devices()[:_NCORES])
    return _pmapped


def kernel(**inputs) -> np.ndarray:
    f = _get_pmapped()
    args = [np.asarray(inputs[n]) for n in _ARG_NAMES]
    # Shard batch: [B, N, ...] -> [8, 1, N, ...] (one element per core).
    args[0] = args[0].reshape(B, 1, N, DIM)
    args[1] = args[1].reshape(B, 1, N, 4)
    args[2] = args[2].reshape(B, 1, N)
    out = f(*args)  # [8, 1, 3]
    return np.asarray(out).reshape(B, 3).astype(np.float32)
